# revision 26
# baseline (speedup 1.0000x reference)
"""GAT forward on 8 Trainium2 NeuronCores — one attention head per core.

Math (per head, all [4096] nodes):
    h   = x @ W                      [N, 128]
    ci  = h @ w_i  (per-node)        [N]
    cj  = h @ w_j  (per-node)        [N]
    e^T[j, i] = exp(leaky_relu(ci[i] + cj[j] + M[j, i]))   (M = 0 / -240 additive
                fp8 mask; masked entries exp to ~1e-21 ~ 0)
    yT[f, i] = sum_j h[j, f] * eT[j, i]        (PE matmul, e as moving operand)
    rs[i]    = sum_j eT[j, i]                  (PE matmul vs ones column)
    y[i, f]  = yT[f, i] / rs[i] + (x @ W_r_head)[i, f]     (+ bias on host)

v4 layout/scheduling notes:
  - Mask DMA'd as fp8 (-240/0, 1 byte/elem); rides the combine STT additively.
  - x / W / Wr stream as bf16; h stationary + e moving are bf16 (1 cyc/col on
    the PE); scores z stay fp32 until the exp.
  - NO GpSimd in the per-tile path: concurrent Pool-engine tensor ops slow
    DVE stt's from ~2.28us to ~5.8us (measured memory-port contention).
  - Per-j-tile routing (period 32, 13x T1 : 19x T2) balances DVE vs ACT:
      T1: DVE combine stt + DVE leaky-relu stt ((z*.2) max z) + ACT exp
      T2: DVE combine stt + ACT prelu (in place) + ACT exp
  - Residual projection (x @ Wr) is DEFERRED out of phase 1: its matmuls
    run during half-0 attention in spare PSUM banks (yT 4 + rs 2 + resid 2),
    off the critical path; phase 1 only builds hT/ci/cj/h (~25us).
  - rowsum matmul chunks packed into 2 PSUM banks via output partitions
    0/32/64 (+1 spill region).
  - i is split in two 2048-wide halves; finales deferred as in the baseline.
"""
import sys

sys.path.insert(0, "/opt/trn_rl_repo")
from contextlib import ExitStack

import numpy as np
import ml_dtypes

import concourse.bass as bass
import concourse.tile as tile
from concourse import bacc, mybir
from concourse.bass_utils import run_bass_kernel_spmd

dt = mybir.dt
F32, F32R, BF16, FP8 = dt.float32, dt.float32r, dt.bfloat16, dt.float8e4
AF = mybir.ActivationFunctionType
OP = mybir.AluOpType

N = 4096
IN_F = 512
HF = 128
HEADS = 8
SLOPE = 0.2
MASK_NEG = -240.0  # most-negative finite fp8e4m3; exp(0.2*(z-240)) ~ 1e-21
HALF = 2048
NJT = N // 128  # 32 j-tiles
NMC = IN_F // 128  # 4 contraction chunks over in-features

# 11x T1 (DVE-heavy) : 21x T2 (ACT-heavy) per 32 j-tiles.
ROUTE = ["1" if (k * 11) % 32 < 11 else "2" for k in range(32)]

_prog = None


def build_program():
    nc = bacc.Bacc("TRN2", target_bir_lowering=False, debug=False)
    xT_d = nc.dram_tensor("xT", [IN_F, N], BF16, kind="ExternalInput").ap()
    mask_d = nc.dram_tensor("mask", [N, N], FP8, kind="ExternalInput").ap()
    W_d = nc.dram_tensor("W", [IN_F, HF], BF16, kind="ExternalInput").ap()
    Wr_d = nc.dram_tensor("Wr", [IN_F, HF], BF16, kind="ExternalInput").ap()
    wi_d = nc.dram_tensor("wi", [HF, 1], F32, kind="ExternalInput").ap()
    wj_d = nc.dram_tensor("wj", [HF, 1], F32, kind="ExternalInput").ap()
    eye_d = nc.dram_tensor("eye", [128, 128], F32, kind="ExternalInput").ap()
    y_d = nc.dram_tensor("y", [N, HF], F32, kind="ExternalOutput").ap()

    with tile.TileContext(nc) as tc, ExitStack() as ctx:
        persist = ctx.enter_context(tc.tile_pool(name="persist", bufs=1))
        h_sb = persist.tile([128, N], BF16, tag="h")  # h[j,f], jt-sliced
        resid_sb = persist.tile([128, N], F32, tag="resid")  # resid[i,f]
        # bf16 ciB is safe: ci is constant per softmax row, so its rounding
        # cancels in the normalization (up to the prelu kink, which is tiny)
        ciB = persist.tile([128, N], BF16, tag="ciB")  # ci broadcast on parts
        cjT = persist.tile([128, 2 * NJT], F32, tag="cjT")  # cj cols (even idx)
        eye_sb = persist.tile([128, 128], F32, tag="eye")
        ones_bf = persist.tile([128, 1], BF16, tag="ones")
        Wr_sb = persist.tile([128, NMC * HF], BF16, tag="Wr")

        nc.sync.dma_start(eye_sb[:], eye_d)
        nc.vector.memset(ones_bf[:], 1.0)
        eye_r = persist.tile([128, 128], F32R, tag="eye_r")
        nc.vector.tensor_copy(eye_r[:], eye_sb[:])
        for mc in range(NMC):
            nc.sync.dma_start(
                Wr_sb[:, mc * HF : (mc + 1) * HF],
                Wr_d[mc * 128 : (mc + 1) * 128, :],
            )

        # Phase-2 pools opened FIRST: disjoint from phase-1 SBUF so attention
        # tiles never wait on projection-buffer releases.
        maskp = ctx.enter_context(tc.tile_pool(name="maskp", bufs=5))
        zpool = ctx.enter_context(tc.tile_pool(name="zpool", bufs=3))
        upool = ctx.enter_context(tc.tile_pool(name="upool", bufs=2))
        epool = ctx.enter_context(tc.tile_pool(name="epool", bufs=7))
        rsp = ctx.enter_context(tc.tile_pool(name="rsp", bufs=1))
        fin = ctx.enter_context(tc.tile_pool(name="fin", bufs=2))
        outp = ctx.enter_context(tc.tile_pool(name="outp", bufs=2))
        # deferred-resid x stream pool (used during half-0 attention)
        xr_pool = ctx.enter_context(tc.tile_pool(name="xr", bufs=2))

        # ---------- Phase 1: hT[f,j], ci, cj, h (resid deferred) ----------
        with ExitStack() as p1:
            ph1 = p1.enter_context(tc.tile_pool(name="ph1", bufs=1))
            xpool = p1.enter_context(tc.tile_pool(name="xpool", bufs=3))
            psb = p1.enter_context(tc.tile_pool(name="psb", bufs=1, space="PSUM"))

            W_sb = ph1.tile([128, NMC * HF], BF16, tag="W")
            for mc in range(NMC):
                nc.sync.dma_start(
                    W_sb[:, mc * HF : (mc + 1) * HF], W_d[mc * 128 : (mc + 1) * 128, :]
                )
            wi_sb = ph1.tile([128, 1], F32, tag="wi")
            nc.sync.dma_start(wi_sb[:], wi_d)
            wj_sb = ph1.tile([128, 1], F32, tag="wj")
            nc.sync.dma_start(wj_sb[:], wj_d)
            wi_r = ph1.tile([128, 1], F32R, tag="wi_r")
            nc.vector.tensor_copy(wi_r[:], wi_sb[:])
            # wj padded to 2 columns: f32r matmuls need an even moving free dim
            wj2_f = ph1.tile([128, 2], F32, tag="wj2_f")
            nc.vector.memset(wj2_f[:], 0.0)
            nc.vector.tensor_copy(wj2_f[:, 0:1], wj_sb[:])
            wj_r = ph1.tile([128, 2], F32R, tag="wj_r")
            nc.vector.tensor_copy(wj_r[:], wj2_f[:])

            hT_sb = ph1.tile([128, N], F32R, tag="hT")  # hT[f, j]

            # mc-outer: 4 big [128, 4096] x-DMAs (fewest DMA setups), both
            # halves' hT accumulating at once (8 PSUM banks)
            ps_hT0 = psb.tile([128, HALF], F32, tag="psA")
            ps_hT1 = psb.tile([128, HALF], F32, tag="psB")
            for mc in range(NMC):
                xt = xpool.tile([128, N], BF16, tag="xt")
                nc.sync.dma_start(xt[:], xT_d[mc * 128 : (mc + 1) * 128, :])
                for hf, ps in ((0, ps_hT0), (1, ps_hT1)):
                    for nck in range(HALF // 512):
                        nc.tensor.matmul(
                            ps[:, nck * 512 : (nck + 1) * 512],
                            W_sb[:, mc * HF : (mc + 1) * HF],
                            xt[:, hf * HALF + nck * 512 : hf * HALF + (nck + 1) * 512],
                            start=(mc == 0),
                            stop=(mc == NMC - 1),
                        )
            for hf, ps in ((0, ps_hT0), (1, ps_hT1)):
                o = hf * HALF
                for nck in range(HALF // 512):
                    nc.vector.tensor_copy(
                        hT_sb[:, o + nck * 512 : o + (nck + 1) * 512],
                        ps[:, nck * 512 : (nck + 1) * 512],
                    )

            for hf in range(2):
                o = hf * HALF
                # ci for this half -> broadcast that half of ciB immediately
                ps_ci = psb.tile([1, HALF], F32, tag="psA")
                for nck in range(HALF // 512):
                    nc.tensor.matmul(
                        ps_ci[0:1, nck * 512 : (nck + 1) * 512],
                        wi_r[:],
                        hT_sb[:, o + nck * 512 : o + (nck + 1) * 512],
                        start=True,
                        stop=True,
                    )
                ci_rowh = ph1.tile([1, HALF], BF16, tag="ci_row")
                nc.vector.tensor_copy(ci_rowh[:], ps_ci[:])
                nc.gpsimd.partition_broadcast(ciB[:, o : o + HALF], ci_rowh[0:1, :])

                # cj columns for this half of j-tiles (reuses the psB slot,
                # free after the hT evacuations)
                ps_cj = psb.tile([128, HALF], F32, tag="psB")
                for k in range(NJT // 2):
                    jt = hf * (NJT // 2) + k
                    nc.tensor.matmul(
                        ps_cj[:, 2 * k : 2 * k + 2],
                        hT_sb[:, jt * 128 : (jt + 1) * 128],
                        wj_r[:],
                        start=(k == 0),
                        stop=(k == NJT // 2 - 1),
                    )
                nc.vector.tensor_copy(
                    cjT[:, hf * NJT : (hf + 1) * NJT], ps_cj[:, 0:NJT]
                )

                # h[j, f] bf16 for this half of j-tiles = transpose(hT)
                ps_h = psb.tile([128, HALF], F32R, tag="psA")
                for k in range(HALF // 128):
                    jt = hf * (HALF // 128) + k
                    nc.tensor.transpose(
                        ps_h[:, k * 128 : (k + 1) * 128],
                        hT_sb[:, jt * 128 : (jt + 1) * 128],
                        eye_r[:],
                    )
                nc.scalar.copy(h_sb[:, o : o + HALF], ps_h[:])

            # deferred resid in the freed psB banks: re-stream x in 512-col
            # blocks; runs in the phase-1 tail, overlapping phase-2 elementwise
            ps_res = psb.tile([128, HALF], F32, tag="psB")
            for rb in range(8):
                ro = (rb % 4) * 512
                for mc in range(NMC):
                    xt = xr_pool.tile([128, 512], BF16, tag="xt")
                    nc.sync.dma_start(
                        xt[:],
                        xT_d[mc * 128 : (mc + 1) * 128, rb * 512 : (rb + 1) * 512],
                    )
                    for it in range(4):
                        nc.tensor.matmul(
                            ps_res[:, ro + it * 128 : ro + (it + 1) * 128],
                            xt[:, it * 128 : (it + 1) * 128],
                            Wr_sb[:, mc * HF : (mc + 1) * HF],
                            start=(mc == 0 and it == 0),
                            stop=(mc == NMC - 1 and it == 3),
                        )
                nc.scalar.copy(
                    resid_sb[:, rb * 512 : (rb + 1) * 512], ps_res[:, ro : ro + 512]
                )
                if rb % 4 == 3:
                    ps_res = psb.tile([128, HALF], F32, tag="psB")

        # ---------- Phase 2: attention (+ deferred resid during half 0) ----
        for half in range(2):
            i0 = half * HALF
            with ExitStack() as pmm_ctx:
                pmm = pmm_ctx.enter_context(
                    tc.tile_pool(name=f"pmm{half}", bufs=1, space="PSUM")
                )
                yT_ps = pmm.tile([128, HALF], F32, tag="yT")
                # rowsum chunks packed by output partition: chunk c<3 at
                # partition 32c cols 0:512, chunk 3 at partition 0 cols 512:1024
                rs_ps = pmm.tile([128, 1024], F32, tag="rs")

                for jt in range(NJT):
                    col = (jt // (NJT // 2)) * NJT + 2 * (jt % (NJT // 2))
                    route = ROUTE[jt % 32]

                    m_t = maskp.tile([128, HALF], FP8, tag="m")
                    nc.sync.dma_start(
                        m_t[:], mask_d[jt * 128 : (jt + 1) * 128, i0 : i0 + HALF]
                    )

                    z_t = zpool.tile([128, HALF], F32, tag="z")
                    nc.vector.scalar_tensor_tensor(
                        z_t[:],
                        ciB[:, i0 : i0 + HALF],
                        cjT[:, col : col + 1],
                        m_t[:],
                        op0=OP.add,
                        op1=OP.add,
                    )
                    if route == "1":
                        u_t = upool.tile([128, HALF], F32, tag="u")
                        nc.vector.scalar_tensor_tensor(
                            u_t[:], z_t[:], SLOPE, z_t[:], op0=OP.mult, op1=OP.max
                        )
                        exp_src = u_t
                    else:
                        nc.scalar.activation(z_t[:], z_t[:], AF.Prelu, alpha=SLOPE)
                        exp_src = z_t

                    e_t = epool.tile([128, HALF], BF16, tag="e")
                    nc.scalar.activation(e_t[:], exp_src[:], AF.Exp)

                    hr = h_sb[:, jt * 128 : (jt + 1) * 128]
                    for c in range(HALF // 512):
                        nc.tensor.matmul(
                            yT_ps[:, c * 512 : (c + 1) * 512],
                            hr,
                            e_t[:, c * 512 : (c + 1) * 512],
                            start=(jt == 0),
                            stop=(jt == NJT - 1),
                        )
                    for c in range(HALF // 512):
                        rs_out = (
                            rs_ps[32 * c : 32 * c + 1, 0:512]
                            if c < 3
                            else rs_ps[0:1, 512:1024]
                        )
                        nc.tensor.matmul(
                            rs_out,
                            ones_bf[:],
                            e_t[:, c * 512 : (c + 1) * 512],
                            start=(jt == 0),
                            stop=(jt == NJT - 1),
                        )

                yT_sb = fin.tile([128, HALF], F32, tag="yT_sb")
                nc.vector.tensor_copy(yT_sb[:], yT_ps[:])
                # gather packed rowsum chunks into one partition-0 row
                rs_sb = rsp.tile([1, HALF], F32, tag="rs_sb")
                for c in range(4):
                    src = (
                        rs_ps[32 * c : 32 * c + 1, 0:512]
                        if c < 3
                        else rs_ps[0:1, 512:1024]
                    )
                    if c % 2 == 0:
                        nc.vector.tensor_copy(rs_sb[0:1, c * 512 : (c + 1) * 512], src)
                    else:
                        nc.scalar.copy(rs_sb[0:1, c * 512 : (c + 1) * 512], src)

            # per-half finale: brief PSUM use between the two halves
            with ExitStack() as pf_ctx:
                pfin = pf_ctx.enter_context(
                    tc.tile_pool(name=f"pfin{half}", bufs=1, space="PSUM")
                )
                rsT_ps = pfin.tile([128, HALF // 128], F32, tag="rsT")
                for g in range(HALF // 128):
                    nc.tensor.transpose(
                        rsT_ps[:, g : g + 1],
                        rs_sb[0:1, g * 128 : (g + 1) * 128],
                        eye_sb[0:1, 0:1],
                    )
                rsT_sb = fin.tile([128, HALF // 128], F32, tag="rsT_sb")
                nc.vector.tensor_copy(rsT_sb[:], rsT_ps[:])
                recipT = fin.tile([128, HALF // 128], F32, tag="recipT")
                nc.vector.reciprocal(recipT[:], rsT_sb[:])

                tr_ps = pfin.tile([128, HALF], F32, tag="tr")
                for gi in range(HALF // 128):
                    nc.tensor.transpose(
                        tr_ps[:, gi * 128 : (gi + 1) * 128],
                        yT_sb[:, gi * 128 : (gi + 1) * 128],
                        eye_sb[:],
                    )
                # evacuate transposed y so PSUM banks free for the next half
                ytr_sb = fin.tile([128, HALF], F32, tag="ytr_sb")
                nc.vector.tensor_copy(ytr_sb[:], tr_ps[:])
            for gi in range(HALF // 128):
                g = half * (HALF // 128) + gi
                ob = outp.tile([128, HF], F32, tag="ob")
                nc.vector.scalar_tensor_tensor(
                    ob[:],
                    ytr_sb[:, gi * 128 : (gi + 1) * 128],
                    recipT[:, gi : gi + 1],
                    resid_sb[:, g * 128 : (g + 1) * 128],
                    op0=OP.mult,
                    op1=OP.add,
                )
                nc.sync.dma_start(y_d[g * 128 : (g + 1) * 128, :], ob[:])

    nc.compile()
    return nc


def _get_program():
    global _prog
    if _prog is None:
        _prog = build_program()
    return _prog


def _prepare_in_maps(x, graph, W, w_i, w_j, W_r):
    xT = np.ascontiguousarray(x.T).astype(ml_dtypes.bfloat16)
    mask = np.where(graph > 0, np.float32(0.0), np.float32(MASK_NEG)).astype(
        ml_dtypes.float8_e4m3
    )
    eye = np.eye(128, dtype=np.float32)
    in_maps = []
    for c in range(HEADS):
        in_maps.append(
            {
                "xT": xT,
                "mask": mask,
                "W": np.ascontiguousarray(W[c]).astype(ml_dtypes.bfloat16),
                "Wr": np.ascontiguousarray(W_r[:, c * HF : (c + 1) * HF]).astype(
                    ml_dtypes.bfloat16
                ),
                "wi": np.ascontiguousarray(w_i[c]).astype(np.float32, copy=False),
                "wj": np.ascontiguousarray(w_j[c]).astype(np.float32, copy=False),
                "eye": eye,
            }
        )
    return in_maps


def run(inputs, trace=False, **kwargs):
    """Run the SPMD kernel; returns (y_full, BassKernelResults)."""
    x = np.asarray(inputs["x"], dtype=np.float32)
    graph = np.asarray(inputs["graph"])
    W = np.asarray(inputs["W"], dtype=np.float32)
    w_i = np.asarray(inputs["w_i"], dtype=np.float32)
    w_j = np.asarray(inputs["w_j"], dtype=np.float32)
    W_r = np.asarray(inputs["W_r"], dtype=np.float32)
    bias = np.asarray(inputs["bias"], dtype=np.float32)

    nc = _get_program()
    in_maps = _prepare_in_maps(x, graph, W, w_i, w_j, W_r)
    br = run_bass_kernel_spmd(
        nc, in_maps, core_ids=list(range(HEADS)), trace=trace, **kwargs
    )
    y = np.concatenate([br.results[c]["y"] for c in range(HEADS)], axis=1)
    y = y + bias[None, :]
    return y.astype(np.float32), br


def kernel(**inputs):
    y, _ = run(inputs)
    return y
